# revision 49
# baseline (speedup 1.0000x reference)
"""Trainium2 Bass kernel v11 for nn_Net_3152505995417 (gnn_message_passing).

Pair-dense closed form (see v2 history), rescheduled from HW traces
(27.4us baseline -> 25.2us):
  - d1 on the PE via one scaled-identity tile scI (p1_k I), 5
    accumulating [116x116] matmuls with lhsT = ea_k (symmetric).
  - ricROW via diag trick: dric = I*ric, ricROW = ONES116 @ dric
    (kills the 0.8us single-partition row reciprocal).
  - u row: srow = ones^T Gn (PE colsum, no Gd prepass), u16 =
    srow * dvrow on a [1,chunk] DVE pass.
  - DMA: HWDGE completion sems fire ~1.9us after ring-issue; first
    DMA on each ring carries the most critical data (sync: ea+consts
    as ONE block; scalar: B1).  s32 alone on gpsimd (larger SWDGE
    transfers trigger a multi-us drain that halves DVE throughput).
  - ACT table warmed behind the DMA phase; eR on scalar; dg chain on
    gpsimd; e2 relu planes split scalar/vector; pU/e2/rs trail in 3
    column chunks; 1/N and bbl folded into y16 scale / PSUM accum.
Replicated on all 8 cores; core 0's output is returned.
"""

import numpy as np

N = 116
E = N * (N - 1) // 2
HID = 64
EDIM = 5
OUT = 4
ENC = HID + N // 2
EPS = 1e-10
SLAB = EDIM * N  # 580

# ---- s16 (bf16) column map ----
# block A (first sync DMA): ea + the constants feeding scI/d1/ric
C_EA = 0                      # [116, 0:580] pair-dense edge attrs, k-major
C_I = 580                     # [116, 580:696] identity
C_ONES2 = 696                 # [116, 696:812] all-ones matrix
C_SVB = 812                   # [116, 812:852] svec bcast (bf16)
C_MASK = 852                  # [116, 852:857] p2 bcast cols
C_ONESC = 857                 # [116, 857:858] ones col
BA_LO, BA_HI = 0, 858
# block B1 (scalar ring): x-chain weights + small rows
C_ENCT = 858                  # [122, 858:974]
C_WENC = 974                  # [122, 974:1038]
C_W1 = 1038                   # [64, 1038:1102]
C_W2T = 1102                  # [64, 1102:1166]  W2 transposed
C_WL = 1166                   # [64, 1166:1170]
C_SV = 1170                   # [1, 1170:1210]  p1|p2|be|We flat
C_ONESR = 1210                # [1, 1210:1326] ones row
C_PET = 1326                  # [64, 1326:1327] pe as column
C_B2 = 1327                   # [64, 1327:1328] b2 as column
C_BLR = 1328                  # [1, 1328:1332] bl as bf16 row
B1_LO, B1_HI = 858, 1332
C_WEX = 1332                  # [116, 1332+580k : ...] We[k,m] bcast, k=0..3
S16_W = C_WEX + 4 * SLAB      # 3652

# s32 (fp32) columns
C32_BENC = 0                  # [64,1]
C32_B1 = 1
C32_BL = 2                    # [4,1]
C32_SV = 4                    # [116, 4:44] p1|p2|be|We broadcast to all rows
S32_W = 44

# 2-chunk split for Gn/S/q/u; 3-chunk split for pU/e2/rs
CH1 = (0, 2 * N)
CH2 = (2 * N, SLAB)
CHUNKS = [(0, 2, CH1), (2, 5, CH2)]
PCHUNKS = [(0, 2, (0, 2 * N)), (2, 4, (2 * N, 4 * N)), (4, 5, (4 * N, SLAB))]

_CACHE = {}


def _split_excess_waits(nc, mybir, max_waits=1):
    """Walrus on this build accepts only one sync-wait per instruction;
    move excess waits onto chained NoOps on the same engine."""
    for fn in nc.m.functions:
        for blk in fn.blocks:
            insts = blk.instructions
            new, changed = [], False
            for ins in insts:
                si = ins.sync_info
                waits = list(si.on_wait) if si is not None else []
                if len(waits) > max_waits:
                    while len(waits) > max_waits:
                        chunk, waits = waits[:1], waits[1:]
                        nop = mybir.InstNoOp(
                            name=nc.get_next_instruction_name(),
                            engine=ins.engine,
                            sync_info=mybir.SyncInfo(on_wait=chunk, on_update=[]),
                            bass_nofuse=True,
                        )
                        new.append(nop)
                    si.on_wait = waits
                    changed = True
                new.append(ins)
            if changed:
                blk.instructions = new


def _build():
    import concourse.bass as bass
    import concourse.tile as tile
    from concourse import mybir

    f32 = mybir.dt.float32
    bf16 = mybir.dt.bfloat16
    A = mybir.AluOpType
    Relu = mybir.ActivationFunctionType.Relu
    Ident = mybir.ActivationFunctionType.Identity

    nc = bass.Bass("TRN2", target_bir_lowering=False, num_devices=8)

    s16_d = nc.declare_dram_parameter("s16", [128, S16_W], bf16, isOutput=False)
    s32_d = nc.declare_dram_parameter("s32", [128, S32_W], f32, isOutput=False)
    out_d = nc.declare_dram_parameter("out", [OUT, 1], f32, isOutput=True)

    with tile.TileContext(nc) as tc:
        with (
            tc.tile_pool(name="sb", bufs=1) as sb,
            tc.tile_pool(name="pm", bufs=4, space="PSUM") as pm,
            tc.tile_pool(name="pu", bufs=1, space="PSUM") as pu,
        ):
            t16 = sb.tile([128, S16_W], bf16, tag="t16")
            t32 = sb.tile([128, S32_W], f32, tag="t32")

            # ---- input DMAs ----
            # sync ring: [ea + constants] as ONE first DMA, then WeX0/1/2
            nc.sync.dma_start(out=t16[:, BA_LO:BA_HI], in_=s16_d[:, BA_LO:BA_HI])
            nc.sync.dma_start(
                out=t16[:, C_WEX:C_WEX + 2 * SLAB],
                in_=s16_d[:, C_WEX:C_WEX + 2 * SLAB],
            )
            nc.sync.dma_start(
                out=t16[:, C_WEX + 2 * SLAB:C_WEX + 3 * SLAB],
                in_=s16_d[:, C_WEX + 2 * SLAB:C_WEX + 3 * SLAB],
            )
            # gpsimd ring: only the tiny s32 block (big SWDGE transfers
            # trigger a long drain that steals DVE throughput)
            nc.gpsimd.dma_start(out=t32[:], in_=s32_d[:])
            # scalar ring: B1 first, then the table warm, then WeX2/3
            nc.scalar.dma_start(out=t16[:, B1_LO:B1_HI], in_=s16_d[:, B1_LO:B1_HI])

            # ---- named views ----
            ea = t16[0:N, C_EA:C_EA + SLAB]
            encT = t16[0:ENC, C_ENCT:C_ENCT + N]
            Wenc = t16[0:ENC, C_WENC:C_WENC + HID]
            W1 = t16[0:HID, C_W1:C_W1 + HID]
            W2T = t16[0:HID, C_W2T:C_W2T + HID]
            Wl = t16[0:HID, C_WL:C_WL + OUT]
            ones_row = t16[0:1, C_ONESR:C_ONESR + N]
            ones_col = t16[0:N, C_ONESC:C_ONESC + 1]
            peT = t16[0:HID, C_PET:C_PET + 1]
            b216 = t16[0:HID, C_B2:C_B2 + 1]
            I116 = t16[0:N, C_I:C_I + N]
            p2c16 = t16[0:N, C_MASK:C_MASK + EDIM]
            ONES2 = t16[0:N, C_ONES2:C_ONES2 + N]
            svb16 = t16[0:N, C_SVB:C_SVB + 40]
            blrow16 = t16[0:1, C_BLR:C_BLR + OUT]
            one11 = t16[0:1, C_ONESR:C_ONESR + 1]

            def WeX(k):
                return t16[0:N, C_WEX + k * SLAB:C_WEX + (k + 1) * SLAB]

            benc = t32[0:HID, C32_BENC:C32_BENC + 1]
            b1 = t32[0:HID, C32_B1:C32_B1 + 1]
            bl = t32[0:OUT, C32_BL:C32_BL + 1]

            svecB = t32[0:N, C32_SV:C32_SV + 40]
            p2B = svecB[:, 5:10]
            beB = svecB[:, 10:15]

            def m3(ap, nm=EDIM):
                return ap.rearrange("p (m j) -> p m j", m=nm)

            # ---- scalar: warm the ACT table (after its DMA issues) ----
            warm_src = sb.tile([1, 1], f32, tag="warm_src")
            nc.vector.memset(warm_src[:], 1.0)
            warm = sb.tile([1, 1], f32, tag="warm")
            nc.scalar.activation(warm[:], warm_src[:], Relu)
            # eR on scalar (vector front stays free for scI/P's)
            eR = sb.tile([N, SLAB], bf16, tag="eR")
            nc.scalar.activation(eR[:], ea, Relu)
            # scalar ring: WeX3 after the table load + eR
            nc.scalar.dma_start(
                out=t16[:, C_WEX + 3 * SLAB:C_WEX + 4 * SLAB],
                in_=s16_d[:, C_WEX + 3 * SLAB:C_WEX + 4 * SLAB],
            )

            # ---- PE first wave: xT (needs only B1 + s32-free) ----
            xT_ps = pm.tile([HID, N], f32, tag="ps")
            nc.tensor.matmul(xT_ps[:], Wenc, encT, start=True, stop=True)

            # ---- vector: scI, xT bias-add ----
            # scI[i,(k,j)] = p1_k * I[i,j]  (all-bf16: 2x DVE mode)
            scI = sb.tile([N, SLAB], bf16, tag="scI")
            nc.vector.tensor_tensor(
                m3(scI[:]),
                I116[:][:, None, :].to_broadcast([N, EDIM, N]),
                svb16[:, 0:5][:, :, None].to_broadcast([N, EDIM, N]),
                A.mult,
            )
            xT = sb.tile([HID, N], bf16, tag="xT")
            nc.vector.tensor_scalar(xT[:], xT_ps[:], benc[:, 0:1], None, A.add)

            # ---- PE second wave: d1 accumulation, then xW1 ----
            # d1 = sum_k ea_k @ (p1_k I) (lhsT = ea_k, symmetric)
            d1_ps = pm.tile([N, N], f32, tag="ps")
            for k in range(EDIM):
                nc.tensor.matmul(
                    d1_ps[:], ea[:, k * N:(k + 1) * N],
                    scI[:, k * N:(k + 1) * N],
                    start=(k == 0), stop=(k == EDIM - 1),
                )
            xW1_ps = pm.tile([N, HID], f32, tag="ps")
            nc.tensor.matmul(xW1_ps[:], xT[:], W1, start=True, stop=True)
            xW1 = sb.tile([N, HID], bf16, tag="xW1")
            nc.scalar.copy(xW1[:], xW1_ps[:])
            d1 = sb.tile([N, N], bf16, tag="d1")
            nc.scalar.copy(d1[:], d1_ps[:])

            def eRb(k):
                return eR[:, k * N:(k + 1) * N][:, None, :].to_broadcast(
                    [N, EDIM, N]
                )

            # ---- vector: P/add tree (P4 first: no WeX dependency) ----
            P4 = sb.tile([N, SLAB], bf16, tag="P4")
            WeB4 = svb16[:, 15 + 4 * 5:15 + 4 * 5 + 5][:, :, None].to_broadcast(
                [N, EDIM, N]
            )
            nc.vector.tensor_tensor(m3(P4[:]), eRb(4), WeB4, A.mult)
            # P products for k=0..3 as TWO 4D-AP passes (k pairs); the
            # m axis broadcasts (mid-dim stride 0 keeps 2x DVE mode)
            Pm = sb.tile([N, 4 * SLAB], bf16, tag="Pm")
            for half in range(2):
                co = C_WEX + half * 2 * SLAB
                eo = half * 2 * N
                nc.vector.tensor_tensor(
                    Pm[:, half * 2 * SLAB:(half + 1) * 2 * SLAB].rearrange(
                        "p (k m j) -> p k m j", k=2, m=EDIM
                    ),
                    eR[:, eo:eo + 2 * N].rearrange(
                        "p (k j) -> p k j", k=2
                    )[:, :, None, :].to_broadcast([N, 2, EDIM, N]),
                    t16[0:N, co:co + 2 * SLAB].rearrange(
                        "p (k m j) -> p k m j", k=2, m=EDIM
                    ),
                    A.mult,
                )
            # A01 | A23 in one pass: sum the two k-planes of each half
            A2 = sb.tile([N, 2 * SLAB], bf16, tag="A2")
            Pm4 = Pm[:].rearrange("p (a b x) -> p a b x", a=2, b=2)
            nc.vector.tensor_tensor(
                A2[:].rearrange("p (a x) -> p a x", a=2),
                Pm4[:, :, 0, :], Pm4[:, :, 1, :], A.add,
            )
            A01 = A2[:, 0:SLAB]
            A23 = A2[:, SLAB:2 * SLAB]

            # ---- x1T chain ----
            x1T_ps = pm.tile([HID, N], f32, tag="ps")
            nc.tensor.matmul(x1T_ps[:], xW1[:], d1[:], start=True, stop=True)
            x1T = sb.tile([HID, N], bf16, tag="x1T")
            nc.scalar.activation(x1T[:], x1T_ps[:], Relu, bias=b1)

            # dv matmuls
            dvT_ps = pm.tile([N, 1], f32, tag="ps")
            nc.tensor.matmul(dvT_ps[:], x1T[:], peT, start=True, stop=True)
            dvr_ps = pm.tile([1, N], f32, tag="ps")
            nc.tensor.matmul(dvr_ps[:], peT, x1T[:], start=True, stop=True)

            dvT = sb.tile([N, 1], f32, tag="dvT")
            nc.scalar.copy(dvT[:], dvT_ps[:])
            negdvT = sb.tile([N, 1], f32, tag="negdvT")
            nc.scalar.mul(negdvT[:], dvT_ps[:], -1.0)
            dvrow16 = sb.tile([1, N], bf16, tag="dvrow16")
            nc.scalar.copy(dvrow16[:], dvr_ps[:])

            # vector: ric chain (before the G adds in queue order)
            c = sb.tile([N, 1], f32, tag="c")
            nc.vector.tensor_scalar(c[:], dvT_ps[:], 0.0, EPS, A.max, A.add)
            ric = sb.tile([N, 1], f32, tag="ric")
            nc.vector.reciprocal(ric[:], c[:])
            dric = sb.tile([N, N], bf16, tag="dric")
            nc.vector.tensor_scalar(dric[:], I116, ric[:, 0:1], None, A.mult)

            # W2l / bb / y (off critical path)
            W2l_ps = pm.tile([HID, OUT], f32, tag="ps")
            nc.tensor.matmul(W2l_ps[:], W2T, Wl, start=True, stop=True)
            W2l = sb.tile([HID, OUT], bf16, tag="W2l")
            nc.scalar.copy(W2l[:], W2l_ps[:])
            bb_ps = pm.tile([1, OUT], f32, tag="ps")
            nc.tensor.matmul(bb_ps[:], b216, Wl, start=True, stop=False)
            nc.tensor.matmul(bb_ps[:], one11, blrow16, start=False, stop=True)
            bblrow = sb.tile([1, OUT], bf16, tag="bblrow")
            nc.scalar.copy(bblrow[:], bb_ps[:])
            y_ps = pm.tile([N, OUT], f32, tag="ps")
            nc.tensor.matmul(y_ps[:], x1T[:], W2l[:], start=True, stop=True)
            y16 = sb.tile([N, OUT], bf16, tag="y16")
            nc.scalar.activation(y16[:], y_ps[:], Ident, scale=1.0 / N)

            # PE: dvROW / ricROW broadcasts
            dvROW_ps = pm.tile([N, N], f32, tag="ps")
            nc.tensor.matmul(dvROW_ps[:], ones_row, dvrow16[:], start=True, stop=True)
            ricROW_ps = pm.tile([N, N], f32, tag="ps")
            nc.tensor.matmul(ricROW_ps[:], ONES2, dric[:], start=True, stop=True)

            # nsd[i,j] = -(dv_i + dv_j) on scalar
            nsd16 = sb.tile([N, N], bf16, tag="nsd16")
            nc.scalar.activation(
                nsd16[:], dvROW_ps[:], Ident, bias=negdvT[:], scale=-1.0
            )

            # ---- vector: r16, finish G ----
            r16 = sb.tile([N, N], bf16, tag="r16")
            nc.vector.tensor_scalar(r16[:], ricROW_ps[:], ric[:], None, A.min)
            G1 = sb.tile([N, SLAB], bf16, tag="G1")
            nc.vector.tensor_tensor(G1[:], A01, A23, A.add)
            G = sb.tile([N, SLAB], bf16, tag="G")
            nc.vector.tensor_tensor(G[:], G1[:], P4[:], A.add)

            Gn = sb.tile([N, SLAB], bf16, tag="Gn")
            q16 = sb.tile([N, SLAB], bf16, tag="q16")
            S = sb.tile([N, EDIM], f32, tag="S")
            t1b = sb.tile([N, EDIM], f32, tag="t1b")
            u16 = sb.tile([1, SLAB], bf16, tag="u16")
            e2 = sb.tile([N, SLAB], bf16, tag="e2")

            srow_ps = []
            for ci, (mlo, mhi, (clo, chi)) in enumerate(CHUNKS):
                nm = mhi - mlo
                sl = slice(clo, chi)
                nc.vector.tensor_tensor(
                    m3(Gn[:, sl], nm), m3(G[:, sl], nm),
                    r16[:][:, None, :].to_broadcast([N, nm, N]), A.mult,
                )
                # PE colsum of Gn chunk: srow[(m,j)] = S[j,m] (symmetry)
                sp = pm.tile([1, chi - clo], f32, tag="ps")
                nc.tensor.matmul(sp[:], ones_col, Gn[:, sl], start=True, stop=True)
                srow_ps.append(sp)
            # u16 then q first (they gate the pU matmuls), S/t1b after
            for ci, (mlo, mhi, (clo, chi)) in enumerate(CHUNKS):
                nm = mhi - mlo
                sl = slice(clo, chi)
                # u16[(m,j)] = dv_j * S[j,m] = srow * dvrow (1-partition)
                nc.vector.tensor_tensor(
                    m3(u16[:, sl], nm),
                    m3(srow_ps[ci][:], nm),
                    dvrow16[:][:, None, :].to_broadcast([1, nm, N]), A.mult,
                )
            nc.vector.tensor_reduce(
                S[:, 0:2][:, :, None], m3(Gn[:, CH1[0]:CH1[1]], 2),
                mybir.AxisListType.X, A.add,
            )
            nc.vector.scalar_tensor_tensor(
                t1b[:, 0:2], S[:, 0:2], dvT[:, 0:1], beB[:, 0:2], A.mult, A.add
            )
            nc.vector.tensor_tensor(
                m3(q16[:, CH1[0]:CH1[1]], 2), m3(Gn[:, CH1[0]:CH1[1]], 2),
                nsd16[:][:, None, :].to_broadcast([N, 2, N]), A.mult,
            )
            nc.vector.tensor_reduce(
                S[:, 2:5][:, :, None], m3(Gn[:, CH2[0]:CH2[1]], 3),
                mybir.AxisListType.X, A.add,
            )
            nc.vector.scalar_tensor_tensor(
                t1b[:, 2:5], S[:, 2:5], dvT[:, 0:1], beB[:, 2:5], A.mult, A.add
            )
            nc.vector.tensor_tensor(
                m3(q16[:, CH2[0]:CH2[1]], 3), m3(Gn[:, CH2[0]:CH2[1]], 3),
                nsd16[:][:, None, :].to_broadcast([N, 3, N]), A.mult,
            )

            # ---- pU per 3 chunks: ones (x) u16  +  I @ q ----
            pUt = [
                pu.tile([N, chi - clo], f32, tag=f"pU{ci}", name=f"pU{ci}")
                for ci, (_, _, (clo, chi)) in enumerate(PCHUNKS)
            ]
            for ci, (mlo, mhi, (clo, chi)) in enumerate(PCHUNKS):
                nc.tensor.matmul(
                    pUt[ci][:], ones_row, u16[:, clo:chi],
                    start=True, stop=False,
                )
                nc.tensor.matmul(
                    pUt[ci][:], I116, q16[:, clo:chi],
                    start=False, stop=True,
                )

            # diag correction dg (gpsimd: all non-ptr ops)
            # (negdg no longer needed; rs16 subtracts dg on vector)
            dvT2 = sb.tile([N, 1], f32, tag="dvT2")
            nc.gpsimd.tensor_scalar(dvT2[:], dvT[:], 2.0, None, A.mult)
            h1a = sb.tile([N, EDIM], f32, tag="h1a")
            nc.gpsimd.tensor_tensor(
                h1a[:], S[:], dvT2[:, 0:1].to_broadcast([N, EDIM]), A.mult
            )
            h1 = sb.tile([N, EDIM], f32, tag="h1")
            nc.gpsimd.tensor_tensor(h1[:], h1a[:], beB, A.add)
            h2 = sb.tile([N, EDIM], f32, tag="h2")
            nc.gpsimd.tensor_scalar(h2[:], h1[:], 0.0, None, A.max)
            dgt = sb.tile([N, EDIM], f32, tag="dgt")
            nc.gpsimd.tensor_tensor(dgt[:], h2[:], p2B, A.mult)
            dg = sb.tile([N, 1], f32, tag="dg")
            nc.vector.tensor_reduce(dg[:], dgt[:], mybir.AxisListType.X, A.add)

            # ---- e2 planes: scalar (m0,m2,m4) + vector (m1,m3); rs accum ----
            rs_ps = pm.tile([N, 1], f32, tag="ps")
            for m in range(EDIM):
                ci = m // 2
                mlo = PCHUNKS[ci][0]
                psl = slice(m * N, (m + 1) * N)
                zsrc = pUt[ci][:, (m - mlo) * N:(m - mlo + 1) * N]
                if m in (0, 3):
                    nc.vector.tensor_scalar(
                        e2[:, psl], zsrc, t1b[:, m:m + 1], 0.0,
                        A.add, A.max,
                    )
                else:
                    nc.scalar.activation(
                        e2[:, psl], zsrc, Relu, bias=t1b[:, m:m + 1]
                    )
                nc.tensor.matmul(
                    rs_ps[:], e2[:, psl], p2c16[:, m:m + 1],
                    start=(m == 0), stop=(m == EDIM - 1),
                )

            # ---- tail: out = (1'(d2*mask) y)/N + bbl (all on scalar) ----
            rs16 = sb.tile([N, 1], bf16, tag="rs16")
            nc.vector.tensor_scalar(rs16[:], rs_ps[:], dg[:], None, A.subtract)
            out4_ps = pm.tile([OUT, 1], f32, tag="ps")
            nc.tensor.matmul(out4_ps[:], y16[:], rs16[:], start=True, stop=False)
            nc.tensor.matmul(out4_ps[:], bblrow[:], one11, start=False, stop=True)
            out_sb = sb.tile([OUT, 1], f32, tag="out_sb")
            nc.vector.tensor_copy(out_sb[:], out4_ps[:])
            nc.sync.dma_start(out=out_d[:], in_=out_sb[:])

    _split_excess_waits(nc, mybir)
    return nc


def _prep_inputs(inputs):
    import ml_dtypes

    bf = ml_dtypes.bfloat16
    ei = np.asarray(inputs["edge_index"][0], dtype=np.int64)
    ej = np.asarray(inputs["edge_index"][1], dtype=np.int64)
    ea = np.asarray(inputs["edge_attr"], dtype=np.float32)

    ea_dense = np.zeros((N, EDIM, N), dtype=np.float32)
    ea_dense[ei, :, ej] = ea
    ea_dense[ej, :, ei] = ea

    svec = np.concatenate(
        [
            np.asarray(inputs["p1"], np.float32).reshape(-1),
            np.asarray(inputs["p2"], np.float32).reshape(-1),
            np.asarray(inputs["be"], np.float32).reshape(-1),
            np.asarray(inputs["We"], np.float32).reshape(-1),
        ]
    )

    s16 = np.zeros((128, S16_W), dtype=bf)
    s16[0:N, C_EA:C_EA + SLAB] = ea_dense.reshape(N, SLAB).astype(bf)
    s16[0:ENC, C_ENCT:C_ENCT + N] = (
        np.asarray(inputs["encoding_raw"], dtype=np.float32).T.astype(bf)
    )
    s16[0:ENC, C_WENC:C_WENC + HID] = np.asarray(
        inputs["W_enc"], dtype=np.float32
    ).astype(bf)
    s16[0:HID, C_W1:C_W1 + HID] = np.asarray(inputs["W1"], np.float32).astype(bf)
    s16[0:HID, C_W2T:C_W2T + HID] = (
        np.asarray(inputs["W2"], np.float32).T.astype(bf)
    )
    s16[0:HID, C_WL:C_WL + OUT] = np.asarray(inputs["Wl"], np.float32).astype(bf)
    s16[0, C_SV:C_SV + 40] = svec.astype(bf)
    s16[0, C_ONESR:C_ONESR + N] = np.ones(N, dtype=bf)
    s16[0:N, C_ONESC] = np.ones(N, dtype=bf)
    s16[0:HID, C_PET] = np.asarray(inputs["pe"], np.float32).reshape(-1).astype(bf)
    s16[0:HID, C_B2] = np.asarray(inputs["b2"], np.float32).reshape(-1).astype(bf)
    s16[0:N, C_I:C_I + N] = np.eye(N, dtype=np.float32).astype(bf)
    p2v = np.asarray(inputs["p2"], np.float32).reshape(-1)
    s16[0:N, C_MASK:C_MASK + EDIM] = np.broadcast_to(p2v[None, :], (N, EDIM)).astype(bf)
    s16[0:N, C_ONES2:C_ONES2 + N] = np.ones((N, N), dtype=np.float32).astype(bf)
    s16[0:N, C_SVB:C_SVB + 40] = np.broadcast_to(svec[None, :], (N, 40)).astype(bf)
    We = np.asarray(inputs["We"], np.float32)  # [5,5] (k, m)
    for k in range(4):
        s16[0:N, C_WEX + k * SLAB:C_WEX + (k + 1) * SLAB] = np.broadcast_to(
            np.repeat(We[k], N)[None, :], (N, SLAB)
        ).astype(bf)

    s32 = np.zeros((128, S32_W), dtype=np.float32)
    s32[0:N, C32_SV:C32_SV + 40] = np.broadcast_to(svec[None, :], (N, 40))
    s32[0:HID, C32_BENC] = np.asarray(inputs["b_enc"], np.float32).reshape(-1)
    s32[0:HID, C32_B1] = np.asarray(inputs["b1"], np.float32).reshape(-1)
    s32[0:OUT, C32_BL] = np.asarray(inputs["bl"], np.float32).reshape(-1)
    s16[0, C_BLR:C_BLR + OUT] = np.asarray(
        inputs["bl"], np.float32
    ).reshape(-1).astype(bf)

    return {"s16": s16, "s32": s32}


def kernel(**inputs) -> np.ndarray:
    import sys

    if "/opt/trn_rl_repo" not in sys.path:
        sys.path.insert(0, "/opt/trn_rl_repo")
    from concourse.bass_utils import run_bass_kernel_spmd

    if "nc" not in _CACHE:
        _CACHE["nc"] = _build()
    nc = _CACHE["nc"]

    in_map = _prep_inputs(inputs)
    res = run_bass_kernel_spmd(
        nc, [in_map] * 8, core_ids=list(range(8)), trace=False
    )
    return np.asarray(res.results[0]["out"], dtype=np.float32).reshape(1, OUT)


# revision 51
# speedup vs baseline: 1.0182x; 1.0182x over previous
"""Trainium2 Bass kernel v11 for nn_Net_3152505995417 (gnn_message_passing).

Pair-dense closed form (see v2 history), rescheduled from HW traces
(27.4us baseline -> 25.2us):
  - d1 on the PE via one scaled-identity tile scI (p1_k I), 5
    accumulating [116x116] matmuls with lhsT = ea_k (symmetric).
  - ricROW via diag trick: dric = I*ric, ricROW = ONES116 @ dric
    (kills the 0.8us single-partition row reciprocal).
  - u row: srow = ones^T Gn (PE colsum, no Gd prepass), u16 =
    srow * dvrow on a [1,chunk] DVE pass.
  - DMA: HWDGE completion sems fire ~1.9us after ring-issue; first
    DMA on each ring carries the most critical data (sync: ea+consts
    as ONE block; scalar: B1).  s32 alone on gpsimd (larger SWDGE
    transfers trigger a multi-us drain that halves DVE throughput).
  - ACT table warmed behind the DMA phase; eR on scalar; dg chain on
    gpsimd; e2 relu planes split scalar/vector; pU/e2/rs trail in 3
    column chunks; 1/N and bbl folded into y16 scale / PSUM accum.
Replicated on all 8 cores; core 0's output is returned.
"""

import numpy as np

N = 116
E = N * (N - 1) // 2
HID = 64
EDIM = 5
OUT = 4
ENC = HID + N // 2
EPS = 1e-10
SLAB = EDIM * N  # 580

# ---- s16 (bf16) column map ----
# block A (first sync DMA): ea + the constants feeding scI/d1/ric
C_EA = 0                      # [116, 0:580] pair-dense edge attrs, k-major
C_I = 580                     # [116, 580:696] identity
C_ONES2 = 696                 # [116, 696:812] all-ones matrix
C_SVB = 812                   # [116, 812:852] svec bcast (bf16)
C_MASK = 852                  # [116, 852:857] p2 bcast cols
C_ONESC = 857                 # [116, 857:858] ones col
BA_LO, BA_HI = 0, 858
# block B1 (scalar ring): x-chain weights + small rows
C_ENCT = 858                  # [122, 858:974]
C_WENC = 974                  # [122, 974:1038]
C_W1 = 1038                   # [64, 1038:1102]
C_W2T = 1102                  # [64, 1102:1166]  W2 transposed
C_WL = 1166                   # [64, 1166:1170]
C_SV = 1170                   # [1, 1170:1210]  p1|p2|be|We flat
C_ONESR = 1210                # [1, 1210:1326] ones row
C_PET = 1326                  # [64, 1326:1327] pe as column
C_B2 = 1327                   # [64, 1327:1328] b2 as column
C_BLR = 1328                  # [1, 1328:1332] bl as bf16 row
B1_LO, B1_HI = 858, 1332
C_WEX = 1332                  # [116, 1332+580k : ...] We[k,m] bcast, k=0..3
S16_W = C_WEX + 4 * SLAB      # 3652

# s32 (fp32) columns
C32_BENC = 0                  # [64,1]
C32_B1 = 1
C32_BL = 2                    # [4,1]
C32_SV = 4                    # [116, 4:44] p1|p2|be|We broadcast to all rows
S32_W = 44

# 2-chunk split for Gn/S/q/u; 3-chunk split for pU/e2/rs
CH1 = (0, 2 * N)
CH2 = (2 * N, SLAB)
CHUNKS = [(0, 2, CH1), (2, 5, CH2)]
PCHUNKS = [(0, 2, (0, 2 * N)), (2, 4, (2 * N, 4 * N)), (4, 5, (4 * N, SLAB))]

_CACHE = {}


def _split_excess_waits(nc, mybir, max_waits=1):
    """Walrus on this build accepts only one sync-wait per instruction;
    move excess waits onto chained NoOps on the same engine."""
    for fn in nc.m.functions:
        for blk in fn.blocks:
            insts = blk.instructions
            new, changed = [], False
            for ins in insts:
                si = ins.sync_info
                waits = list(si.on_wait) if si is not None else []
                if len(waits) > max_waits:
                    while len(waits) > max_waits:
                        chunk, waits = waits[:1], waits[1:]
                        nop = mybir.InstNoOp(
                            name=nc.get_next_instruction_name(),
                            engine=ins.engine,
                            sync_info=mybir.SyncInfo(on_wait=chunk, on_update=[]),
                            bass_nofuse=True,
                        )
                        new.append(nop)
                    si.on_wait = waits
                    changed = True
                new.append(ins)
            if changed:
                blk.instructions = new


def _build():
    import concourse.bass as bass
    import concourse.tile as tile
    from concourse import mybir

    f32 = mybir.dt.float32
    bf16 = mybir.dt.bfloat16
    A = mybir.AluOpType
    Relu = mybir.ActivationFunctionType.Relu
    Ident = mybir.ActivationFunctionType.Identity

    nc = bass.Bass("TRN2", target_bir_lowering=False, num_devices=8)

    s16_d = nc.declare_dram_parameter("s16", [128, S16_W], bf16, isOutput=False)
    s32_d = nc.declare_dram_parameter("s32", [128, S32_W], f32, isOutput=False)
    out_d = nc.declare_dram_parameter("out", [OUT, 1], f32, isOutput=True)

    with tile.TileContext(nc) as tc:
        with (
            tc.tile_pool(name="sb", bufs=1) as sb,
            tc.tile_pool(name="pm", bufs=4, space="PSUM") as pm,
            tc.tile_pool(name="pu", bufs=1, space="PSUM") as pu,
        ):
            t16 = sb.tile([128, S16_W], bf16, tag="t16")
            t32 = sb.tile([128, S32_W], f32, tag="t32")

            # ---- input DMAs ----
            # sync ring: [ea + constants] as ONE first DMA, then WeX0/1/2
            nc.sync.dma_start(out=t16[:, BA_LO:BA_HI], in_=s16_d[:, BA_LO:BA_HI])
            nc.sync.dma_start(
                out=t16[:, C_WEX:C_WEX + SLAB],
                in_=s16_d[:, C_WEX:C_WEX + SLAB],
            )
            nc.sync.dma_start(
                out=t16[:, C_WEX + SLAB:C_WEX + 2 * SLAB],
                in_=s16_d[:, C_WEX + SLAB:C_WEX + 2 * SLAB],
            )
            nc.sync.dma_start(
                out=t16[:, C_WEX + 2 * SLAB:C_WEX + 3 * SLAB],
                in_=s16_d[:, C_WEX + 2 * SLAB:C_WEX + 3 * SLAB],
            )
            # gpsimd ring: only the tiny s32 block (big SWDGE transfers
            # trigger a long drain that steals DVE throughput)
            nc.gpsimd.dma_start(out=t32[:], in_=s32_d[:])
            # scalar ring: B1 first, then the table warm, then WeX2/3
            nc.scalar.dma_start(out=t16[:, B1_LO:B1_HI], in_=s16_d[:, B1_LO:B1_HI])

            # ---- named views ----
            ea = t16[0:N, C_EA:C_EA + SLAB]
            encT = t16[0:ENC, C_ENCT:C_ENCT + N]
            Wenc = t16[0:ENC, C_WENC:C_WENC + HID]
            W1 = t16[0:HID, C_W1:C_W1 + HID]
            W2T = t16[0:HID, C_W2T:C_W2T + HID]
            Wl = t16[0:HID, C_WL:C_WL + OUT]
            ones_row = t16[0:1, C_ONESR:C_ONESR + N]
            ones_col = t16[0:N, C_ONESC:C_ONESC + 1]
            peT = t16[0:HID, C_PET:C_PET + 1]
            b216 = t16[0:HID, C_B2:C_B2 + 1]
            I116 = t16[0:N, C_I:C_I + N]
            p2c16 = t16[0:N, C_MASK:C_MASK + EDIM]
            ONES2 = t16[0:N, C_ONES2:C_ONES2 + N]
            svb16 = t16[0:N, C_SVB:C_SVB + 40]
            blrow16 = t16[0:1, C_BLR:C_BLR + OUT]
            one11 = t16[0:1, C_ONESR:C_ONESR + 1]

            def WeX(k):
                return t16[0:N, C_WEX + k * SLAB:C_WEX + (k + 1) * SLAB]

            benc = t32[0:HID, C32_BENC:C32_BENC + 1]
            b1 = t32[0:HID, C32_B1:C32_B1 + 1]
            bl = t32[0:OUT, C32_BL:C32_BL + 1]

            svecB = t32[0:N, C32_SV:C32_SV + 40]
            p2B = svecB[:, 5:10]
            beB = svecB[:, 10:15]

            def m3(ap, nm=EDIM):
                return ap.rearrange("p (m j) -> p m j", m=nm)

            # ---- scalar: warm the ACT table (after its DMA issues) ----
            warm_src = sb.tile([1, 1], f32, tag="warm_src")
            nc.vector.memset(warm_src[:], 1.0)
            warm = sb.tile([1, 1], f32, tag="warm")
            nc.scalar.activation(warm[:], warm_src[:], Relu)
            # WeX3 DMA issued before eR: the issue slot is dead time on
            # the scalar queue (waiting for the BA sem), and it pulls
            # WeX3's completion sem from ~13.7us to ~11.9us (P3 gate)
            nc.scalar.dma_start(
                out=t16[:, C_WEX + 3 * SLAB:C_WEX + 4 * SLAB],
                in_=s16_d[:, C_WEX + 3 * SLAB:C_WEX + 4 * SLAB],
            )
            # eR on scalar (vector front stays free for scI/P's)
            eR = sb.tile([N, SLAB], bf16, tag="eR")
            nc.scalar.activation(eR[:], ea, Relu)

            # ---- PE first wave: xT (needs only B1 + s32-free) ----
            xT_ps = pm.tile([HID, N], f32, tag="ps")
            nc.tensor.matmul(xT_ps[:], Wenc, encT, start=True, stop=True)

            # ---- vector: scI, xT bias-add ----
            # scI[i,(k,j)] = p1_k * I[i,j]  (all-bf16: 2x DVE mode)
            scI = sb.tile([N, SLAB], bf16, tag="scI")
            nc.vector.tensor_tensor(
                m3(scI[:]),
                I116[:][:, None, :].to_broadcast([N, EDIM, N]),
                svb16[:, 0:5][:, :, None].to_broadcast([N, EDIM, N]),
                A.mult,
            )
            xT = sb.tile([HID, N], bf16, tag="xT")
            nc.scalar.activation(xT[:], xT_ps[:], Ident, bias=benc)

            # ---- PE second wave: d1 accumulation, then xW1 ----
            # d1 = sum_k ea_k @ (p1_k I) (lhsT = ea_k, symmetric)
            d1_ps = pm.tile([N, N], f32, tag="ps")
            for k in range(EDIM):
                nc.tensor.matmul(
                    d1_ps[:], ea[:, k * N:(k + 1) * N],
                    scI[:, k * N:(k + 1) * N],
                    start=(k == 0), stop=(k == EDIM - 1),
                )
            xW1_ps = pm.tile([N, HID], f32, tag="ps")
            nc.tensor.matmul(xW1_ps[:], xT[:], W1, start=True, stop=True)
            xW1 = sb.tile([N, HID], bf16, tag="xW1")
            nc.scalar.copy(xW1[:], xW1_ps[:])
            d1 = sb.tile([N, N], bf16, tag="d1")
            nc.scalar.copy(d1[:], d1_ps[:])

            def eRb(k):
                return eR[:, k * N:(k + 1) * N][:, None, :].to_broadcast(
                    [N, EDIM, N]
                )

            # ---- vector: P/add tree (P4 first: no WeX dependency) ----
            P4 = sb.tile([N, SLAB], bf16, tag="P4")
            WeB4 = svb16[:, 15 + 4 * 5:15 + 4 * 5 + 5][:, :, None].to_broadcast(
                [N, EDIM, N]
            )
            nc.vector.tensor_tensor(m3(P4[:]), eRb(4), WeB4, A.mult)
            P = [sb.tile([N, SLAB], bf16, tag=f"P{k}", name=f"P{k}")
                 for k in range(4)]
            nc.vector.tensor_tensor(m3(P[0][:]), eRb(0), m3(WeX(0)), A.mult)
            nc.vector.tensor_tensor(m3(P[1][:]), eRb(1), m3(WeX(1)), A.mult)
            A01 = sb.tile([N, SLAB], bf16, tag="A01")
            nc.vector.tensor_tensor(A01[:], P[0][:], P[1][:], A.add)
            nc.vector.tensor_tensor(m3(P[2][:]), eRb(2), m3(WeX(2)), A.mult)
            nc.vector.tensor_tensor(m3(P[3][:]), eRb(3), m3(WeX(3)), A.mult)
            A23 = sb.tile([N, SLAB], bf16, tag="A23")
            nc.vector.tensor_tensor(A23[:], P[2][:], P[3][:], A.add)

            # ---- x1T chain ----
            x1T_ps = pm.tile([HID, N], f32, tag="ps")
            nc.tensor.matmul(x1T_ps[:], xW1[:], d1[:], start=True, stop=True)
            x1T = sb.tile([HID, N], bf16, tag="x1T")
            nc.scalar.activation(x1T[:], x1T_ps[:], Relu, bias=b1)

            # dv matmuls
            dvT_ps = pm.tile([N, 1], f32, tag="ps")
            nc.tensor.matmul(dvT_ps[:], x1T[:], peT, start=True, stop=True)
            dvr_ps = pm.tile([1, N], f32, tag="ps")
            nc.tensor.matmul(dvr_ps[:], peT, x1T[:], start=True, stop=True)

            dvT = sb.tile([N, 1], f32, tag="dvT")
            nc.scalar.copy(dvT[:], dvT_ps[:])
            negdvT = sb.tile([N, 1], f32, tag="negdvT")
            nc.scalar.mul(negdvT[:], dvT_ps[:], -1.0)
            dvrow16 = sb.tile([1, N], bf16, tag="dvrow16")
            nc.scalar.copy(dvrow16[:], dvr_ps[:])

            # vector: ric chain (before the G adds in queue order)
            c = sb.tile([N, 1], f32, tag="c")
            nc.vector.tensor_scalar(c[:], dvT_ps[:], 0.0, EPS, A.max, A.add)
            ric = sb.tile([N, 1], f32, tag="ric")
            nc.vector.reciprocal(ric[:], c[:])
            dric = sb.tile([N, N], bf16, tag="dric")
            nc.vector.tensor_scalar(dric[:], I116, ric[:, 0:1], None, A.mult)

            # W2l / bb / y (off critical path)
            W2l_ps = pm.tile([HID, OUT], f32, tag="ps")
            nc.tensor.matmul(W2l_ps[:], W2T, Wl, start=True, stop=True)
            W2l = sb.tile([HID, OUT], bf16, tag="W2l")
            nc.scalar.copy(W2l[:], W2l_ps[:])
            bb_ps = pm.tile([1, OUT], f32, tag="ps")
            nc.tensor.matmul(bb_ps[:], b216, Wl, start=True, stop=False)
            nc.tensor.matmul(bb_ps[:], one11, blrow16, start=False, stop=True)
            bblrow = sb.tile([1, OUT], bf16, tag="bblrow")
            nc.scalar.copy(bblrow[:], bb_ps[:])
            y_ps = pm.tile([N, OUT], f32, tag="ps")
            nc.tensor.matmul(y_ps[:], x1T[:], W2l[:], start=True, stop=True)
            y16 = sb.tile([N, OUT], bf16, tag="y16")
            nc.scalar.activation(y16[:], y_ps[:], Ident, scale=1.0 / N)

            # PE: dvROW / ricROW broadcasts
            dvROW_ps = pm.tile([N, N], f32, tag="ps")
            nc.tensor.matmul(dvROW_ps[:], ones_row, dvrow16[:], start=True, stop=True)
            ricROW_ps = pm.tile([N, N], f32, tag="ps")
            nc.tensor.matmul(ricROW_ps[:], ONES2, dric[:], start=True, stop=True)

            # nsd[i,j] = -(dv_i + dv_j) on scalar
            nsd16 = sb.tile([N, N], bf16, tag="nsd16")
            nc.scalar.activation(
                nsd16[:], dvROW_ps[:], Ident, bias=negdvT[:], scale=-1.0
            )

            # ---- vector: r16, finish G ----
            r16 = sb.tile([N, N], bf16, tag="r16")
            nc.vector.tensor_scalar(r16[:], ricROW_ps[:], ric[:], None, A.min)
            G1 = sb.tile([N, SLAB], bf16, tag="G1")
            nc.vector.tensor_tensor(G1[:], A01[:], A23[:], A.add)
            G = sb.tile([N, SLAB], bf16, tag="G")
            nc.vector.tensor_tensor(G[:], G1[:], P4[:], A.add)

            Gn = sb.tile([N, SLAB], bf16, tag="Gn")
            q16 = sb.tile([N, SLAB], bf16, tag="q16")
            S = sb.tile([N, EDIM], f32, tag="S")
            t1b = sb.tile([N, EDIM], f32, tag="t1b")
            u16 = sb.tile([1, SLAB], bf16, tag="u16")
            e2 = sb.tile([N, SLAB], bf16, tag="e2")

            srow_ps = []
            for ci, (mlo, mhi, (clo, chi)) in enumerate(CHUNKS):
                nm = mhi - mlo
                sl = slice(clo, chi)
                nc.vector.tensor_tensor(
                    m3(Gn[:, sl], nm), m3(G[:, sl], nm),
                    r16[:][:, None, :].to_broadcast([N, nm, N]), A.mult,
                )
                # PE colsum of Gn chunk: srow[(m,j)] = S[j,m] (symmetry)
                sp = pm.tile([1, chi - clo], f32, tag="ps")
                nc.tensor.matmul(sp[:], ones_col, Gn[:, sl], start=True, stop=True)
                srow_ps.append(sp)
            # u16 then q first (they gate the pU matmuls), S/t1b after
            for ci, (mlo, mhi, (clo, chi)) in enumerate(CHUNKS):
                nm = mhi - mlo
                sl = slice(clo, chi)
                # u16[(m,j)] = dv_j * S[j,m] = srow * dvrow (1-partition)
                nc.vector.tensor_tensor(
                    m3(u16[:, sl], nm),
                    m3(srow_ps[ci][:], nm),
                    dvrow16[:][:, None, :].to_broadcast([1, nm, N]), A.mult,
                )
            nc.vector.tensor_reduce(
                S[:, 0:2][:, :, None], m3(Gn[:, CH1[0]:CH1[1]], 2),
                mybir.AxisListType.X, A.add,
            )
            nc.vector.scalar_tensor_tensor(
                t1b[:, 0:2], S[:, 0:2], dvT[:, 0:1], beB[:, 0:2], A.mult, A.add
            )
            nc.vector.tensor_tensor(
                m3(q16[:, CH1[0]:CH1[1]], 2), m3(Gn[:, CH1[0]:CH1[1]], 2),
                nsd16[:][:, None, :].to_broadcast([N, 2, N]), A.mult,
            )
            nc.vector.tensor_reduce(
                S[:, 2:5][:, :, None], m3(Gn[:, CH2[0]:CH2[1]], 3),
                mybir.AxisListType.X, A.add,
            )
            nc.vector.scalar_tensor_tensor(
                t1b[:, 2:5], S[:, 2:5], dvT[:, 0:1], beB[:, 2:5], A.mult, A.add
            )
            nc.vector.tensor_tensor(
                m3(q16[:, CH2[0]:CH2[1]], 3), m3(Gn[:, CH2[0]:CH2[1]], 3),
                nsd16[:][:, None, :].to_broadcast([N, 3, N]), A.mult,
            )

            # ---- pU per 3 chunks: ones (x) u16  +  I @ q ----
            pUt = [
                pu.tile([N, chi - clo], f32, tag=f"pU{ci}", name=f"pU{ci}")
                for ci, (_, _, (clo, chi)) in enumerate(PCHUNKS)
            ]
            for ci, (mlo, mhi, (clo, chi)) in enumerate(PCHUNKS):
                nc.tensor.matmul(
                    pUt[ci][:], ones_row, u16[:, clo:chi],
                    start=True, stop=False,
                )
                nc.tensor.matmul(
                    pUt[ci][:], I116, q16[:, clo:chi],
                    start=False, stop=True,
                )

            # diag correction dg (gpsimd: all non-ptr ops)
            # (negdg no longer needed; rs16 subtracts dg on vector)
            dvT2 = sb.tile([N, 1], f32, tag="dvT2")
            nc.gpsimd.tensor_scalar(dvT2[:], dvT[:], 2.0, None, A.mult)
            h1a = sb.tile([N, EDIM], f32, tag="h1a")
            nc.gpsimd.tensor_tensor(
                h1a[:], S[:], dvT2[:, 0:1].to_broadcast([N, EDIM]), A.mult
            )
            h1 = sb.tile([N, EDIM], f32, tag="h1")
            nc.gpsimd.tensor_tensor(h1[:], h1a[:], beB, A.add)
            h2 = sb.tile([N, EDIM], f32, tag="h2")
            nc.gpsimd.tensor_scalar(h2[:], h1[:], 0.0, None, A.max)
            dgt = sb.tile([N, EDIM], f32, tag="dgt")
            nc.gpsimd.tensor_tensor(dgt[:], h2[:], p2B, A.mult)
            dg = sb.tile([N, 1], f32, tag="dg")
            nc.vector.tensor_reduce(dg[:], dgt[:], mybir.AxisListType.X, A.add)

            # ---- e2 planes: scalar (m0,m2,m4) + vector (m1,m3); rs accum ----
            rs_ps = pm.tile([N, 1], f32, tag="ps")
            for m in range(EDIM):
                ci = m // 2
                mlo = PCHUNKS[ci][0]
                psl = slice(m * N, (m + 1) * N)
                zsrc = pUt[ci][:, (m - mlo) * N:(m - mlo + 1) * N]
                if m in (0, 3):
                    nc.vector.tensor_scalar(
                        e2[:, psl], zsrc, t1b[:, m:m + 1], 0.0,
                        A.add, A.max,
                    )
                else:
                    nc.scalar.activation(
                        e2[:, psl], zsrc, Relu, bias=t1b[:, m:m + 1]
                    )
                nc.tensor.matmul(
                    rs_ps[:], e2[:, psl], p2c16[:, m:m + 1],
                    start=(m == 0), stop=(m == EDIM - 1),
                )

            # ---- tail: out = (1'(d2*mask) y)/N + bbl (all on scalar) ----
            rs16 = sb.tile([N, 1], bf16, tag="rs16")
            nc.vector.tensor_scalar(rs16[:], rs_ps[:], dg[:], None, A.subtract)
            out4_ps = pm.tile([OUT, 1], f32, tag="ps")
            nc.tensor.matmul(out4_ps[:], y16[:], rs16[:], start=True, stop=False)
            nc.tensor.matmul(out4_ps[:], bblrow[:], one11, start=False, stop=True)
            out_sb = sb.tile([OUT, 1], f32, tag="out_sb")
            nc.vector.tensor_copy(out_sb[:], out4_ps[:])
            nc.sync.dma_start(out=out_d[:], in_=out_sb[:])

    _split_excess_waits(nc, mybir)
    return nc


def _prep_inputs(inputs):
    import ml_dtypes

    bf = ml_dtypes.bfloat16
    ei = np.asarray(inputs["edge_index"][0], dtype=np.int64)
    ej = np.asarray(inputs["edge_index"][1], dtype=np.int64)
    ea = np.asarray(inputs["edge_attr"], dtype=np.float32)

    ea_dense = np.zeros((N, EDIM, N), dtype=np.float32)
    ea_dense[ei, :, ej] = ea
    ea_dense[ej, :, ei] = ea

    svec = np.concatenate(
        [
            np.asarray(inputs["p1"], np.float32).reshape(-1),
            np.asarray(inputs["p2"], np.float32).reshape(-1),
            np.asarray(inputs["be"], np.float32).reshape(-1),
            np.asarray(inputs["We"], np.float32).reshape(-1),
        ]
    )

    s16 = np.zeros((128, S16_W), dtype=bf)
    s16[0:N, C_EA:C_EA + SLAB] = ea_dense.reshape(N, SLAB).astype(bf)
    s16[0:ENC, C_ENCT:C_ENCT + N] = (
        np.asarray(inputs["encoding_raw"], dtype=np.float32).T.astype(bf)
    )
    s16[0:ENC, C_WENC:C_WENC + HID] = np.asarray(
        inputs["W_enc"], dtype=np.float32
    ).astype(bf)
    s16[0:HID, C_W1:C_W1 + HID] = np.asarray(inputs["W1"], np.float32).astype(bf)
    s16[0:HID, C_W2T:C_W2T + HID] = (
        np.asarray(inputs["W2"], np.float32).T.astype(bf)
    )
    s16[0:HID, C_WL:C_WL + OUT] = np.asarray(inputs["Wl"], np.float32).astype(bf)
    s16[0, C_SV:C_SV + 40] = svec.astype(bf)
    s16[0, C_ONESR:C_ONESR + N] = np.ones(N, dtype=bf)
    s16[0:N, C_ONESC] = np.ones(N, dtype=bf)
    s16[0:HID, C_PET] = np.asarray(inputs["pe"], np.float32).reshape(-1).astype(bf)
    s16[0:HID, C_B2] = np.asarray(inputs["b2"], np.float32).reshape(-1).astype(bf)
    s16[0:N, C_I:C_I + N] = np.eye(N, dtype=np.float32).astype(bf)
    p2v = np.asarray(inputs["p2"], np.float32).reshape(-1)
    s16[0:N, C_MASK:C_MASK + EDIM] = np.broadcast_to(p2v[None, :], (N, EDIM)).astype(bf)
    s16[0:N, C_ONES2:C_ONES2 + N] = np.ones((N, N), dtype=np.float32).astype(bf)
    s16[0:N, C_SVB:C_SVB + 40] = np.broadcast_to(svec[None, :], (N, 40)).astype(bf)
    We = np.asarray(inputs["We"], np.float32)  # [5,5] (k, m)
    for k in range(4):
        s16[0:N, C_WEX + k * SLAB:C_WEX + (k + 1) * SLAB] = np.broadcast_to(
            np.repeat(We[k], N)[None, :], (N, SLAB)
        ).astype(bf)

    s32 = np.zeros((128, S32_W), dtype=np.float32)
    s32[0:N, C32_SV:C32_SV + 40] = np.broadcast_to(svec[None, :], (N, 40))
    s32[0:HID, C32_BENC] = np.asarray(inputs["b_enc"], np.float32).reshape(-1)
    s32[0:HID, C32_B1] = np.asarray(inputs["b1"], np.float32).reshape(-1)
    s32[0:OUT, C32_BL] = np.asarray(inputs["bl"], np.float32).reshape(-1)
    s16[0, C_BLR:C_BLR + OUT] = np.asarray(
        inputs["bl"], np.float32
    ).reshape(-1).astype(bf)

    return {"s16": s16, "s32": s32}


def kernel(**inputs) -> np.ndarray:
    import sys

    if "/opt/trn_rl_repo" not in sys.path:
        sys.path.insert(0, "/opt/trn_rl_repo")
    from concourse.bass_utils import run_bass_kernel_spmd

    if "nc" not in _CACHE:
        _CACHE["nc"] = _build()
    nc = _CACHE["nc"]

    in_map = _prep_inputs(inputs)
    res = run_bass_kernel_spmd(
        nc, [in_map] * 8, core_ids=list(range(8)), trace=False
    )
    return np.asarray(res.results[0]["out"], dtype=np.float32).reshape(1, OUT)
